# revision 7
# baseline (speedup 1.0000x reference)
import math
import sys

sys.path.insert(0, "/opt/trn_rl_repo")

import numpy as np

# Problem constants (hardcoded per spec)
NQ = 12
SEQ = 16
DD = 3
DIM = 1 << NQ
B_FULL = 2048
N_CORES = 8
B_LOC = B_FULL // N_CORES  # 256 samples per core
P = 128                    # partition tile (samples per tile)

_CACHE = {}


def _pbcast(bass, ap, prt):
    """Broadcast a DRAM tensor (no partition dim) across prt partitions."""
    return bass.AP(tensor=ap.tensor, offset=ap.offset, ap=[[0, prt]] + [list(d) for d in ap.ap])


def _bcast_free(bass, ap, pos, count):
    """Insert a [0, count] broadcast dim at position pos of an AP."""
    dims = [list(d) for d in ap.ap]
    dims.insert(pos, [0, count])
    return bass.AP(tensor=ap.tensor, offset=ap.offset, ap=dims)


def _halves(t, j, nq):
    """psi0/psi1 views of state tile [prt, 2^nq] for a gate on wire j."""
    s = 1 << (nq - 1 - j)
    if s == (1 << (nq - 1)):
        return t[:, :s], t[:, s:]
    v = t.rearrange("p (nb two s) -> p nb two s", two=2, s=s)
    return v[:, :, 0, :], v[:, :, 1, :]


def _cnot_xy(t, c, tq, nq):
    """X={ctrl=1,tgt=0}, Y={ctrl=1,tgt=1} views of state tile for CNOT(c,tq)."""
    dim = 1 << nq
    sc = 1 << (nq - 1 - c)
    st = 1 << (nq - 1 - tq)
    if c < tq:
        a = dim // (2 * sc)
        m = sc // (2 * st)
        v = t.rearrange(
            "p (a twoc m twot s) -> p a twoc m twot s", twoc=2, twot=2, s=st, m=m, a=a
        )
        return v[:, :, 1, :, 0, :], v[:, :, 1, :, 1, :]
    else:
        # c = nq-1 (sc=1), tq = 0 (st = dim/2)
        m = dim // 4
        v = t.rearrange("p (twot m twoc) -> p twot m twoc", twot=2, twoc=2)
        return v[:, 0, :, 1], v[:, 1, :, 1]


def build_program(nq=NQ, seq=SEQ, b_loc=B_LOC, n_cores=N_CORES):
    """Build and compile the per-core SPMD Bass program."""
    key = (nq, seq, b_loc, n_cores)
    if key in _CACHE:
        return _CACHE[key]

    import concourse.bass as bass
    import concourse.bacc as bacc
    import concourse.tile as tile
    from concourse import mybir

    FP = mybir.dt.float32
    AF = mybir.ActivationFunctionType
    ALU = mybir.AluOpType
    AX = mybir.AxisListType.X

    dim = 1 << nq
    prt = min(P, b_loc)
    n_tiles = max(1, b_loc // prt)
    ng = seq * nq

    nc = bacc.Bacc("TRN2", target_bir_lowering=False, debug=False, num_devices=n_cores)
    x_ext = nc.dram_tensor("x", [b_loc, seq, DD], FP, kind="ExternalInput").ap()
    w_ext = nc.dram_tensor("w", [seq, nq, 2 * DD], FP, kind="ExternalInput").ap()
    b_ext = nc.dram_tensor("b", [seq, nq, 2], FP, kind="ExternalInput").ap()
    y_ext = nc.dram_tensor("y", [b_loc, 3], FP, kind="ExternalOutput").ap()

    with tile.TileContext(nc) as tc:
        with (
            tc.tile_pool(name="state", bufs=1) as st,
            tc.tile_pool(name="scal", bufs=1) as sc,
            tc.tile_pool(name="tmp", bufs=2) as tp,
        ):
            for tidx in range(n_tiles):
                stt = nc.vector.scalar_tensor_tensor
                tt = nc.vector.tensor_tensor

                # ---------- inputs ----------
                xt = sc.tile([prt, seq, DD], FP, tag=f"xt{tidx}")
                nc.sync.dma_start(out=xt, in_=x_ext[tidx * prt:(tidx + 1) * prt])
                wrep = sc.tile([prt, seq, nq, 2 * DD], FP, tag=f"wrep{tidx}")
                nc.sync.dma_start(out=wrep, in_=_pbcast(bass, w_ext, prt))
                brep = sc.tile([prt, seq, nq, 2], FP, tag=f"brep{tidx}")
                nc.sync.dma_start(out=brep, in_=_pbcast(bass, b_ext, prt))
                # halve once: angles enter as theta/2 everywhere
                nc.vector.tensor_scalar_mul(wrep, wrep, 0.5)
                nc.vector.tensor_scalar_mul(brep, brep, 0.5)

                # ---------- angles: h = 0.5*(x . W) + 0.5*bias ----------
                xrep = sc.tile([prt, seq, nq, DD], FP, tag=f"xrep{tidx}")
                for j in range(nq):
                    nc.vector.tensor_copy(xrep[:, :, j, :], xt)
                h = []
                for half in range(2):
                    prod = tp.tile([prt, seq, nq, DD], FP, tag="prod")
                    tt(prod, xrep, wrep[:, :, :, half * DD:(half + 1) * DD], ALU.mult)
                    hv = sc.tile([prt, seq, nq], FP, tag=f"h{half}_{tidx}")
                    nc.vector.tensor_reduce(hv, prod, axis=AX, op=ALU.add)
                    tt(hv, hv, brep[:, :, :, half], ALU.add)
                    h.append(hv)

                # ---------- per-gate scalars ----------
                # t = tan(h1), w = -tan(h2); per-step renorm g = prod_j cos(h1)cos(h2)
                pihalf = sc.tile([prt, 1], FP, tag=f"pihalf{tidx}")
                nc.vector.memset(pihalf, math.pi / 2)

                MAGIC = 1.5 * (2.0 ** 23)  # fp32 round-to-nearest-int trick
                TWO_PI = 2.0 * math.pi

                def trig(hv, tag):
                    # sin/cos of unbounded h via range reduction to [-pi, pi]
                    cv = sc.tile([prt, ng], FP, tag=f"c{tag}")
                    sv = tp.tile([prt, ng], FP, tag="sv")
                    hf = hv.rearrange("p a b -> p (a b)")
                    m = tp.tile([prt, ng], FP, tag="m")
                    nc.vector.tensor_scalar(m, hf, 1.0 / TWO_PI, None, ALU.mult)
                    k = tp.tile([prt, ng], FP, tag="k")
                    r = tp.tile([prt, ng], FP, tag="r")
                    # sin: r = m - round(m); x = 2*pi*r
                    nc.vector.tensor_scalar(k, m, MAGIC, MAGIC, ALU.add, ALU.subtract)
                    tt(r, m, k, ALU.subtract)
                    nc.vector.tensor_scalar(r, r, TWO_PI, None, ALU.mult)
                    nc.scalar.activation(sv, r, AF.Sin)
                    # cos: shift phase by +pi/2 (m + 0.25 turns)
                    mc = tp.tile([prt, ng], FP, tag="mc")
                    nc.vector.tensor_scalar(mc, m, 0.25, None, ALU.add)
                    nc.vector.tensor_scalar(k, mc, MAGIC, MAGIC, ALU.add, ALU.subtract)
                    tt(r, mc, k, ALU.subtract)
                    nc.vector.tensor_scalar(r, r, TWO_PI, None, ALU.mult)
                    nc.scalar.activation(cv, r, AF.Sin)
                    rcv = tp.tile([prt, ng], FP, tag="rcv")
                    nc.vector.reciprocal(rcv, cv)
                    dv = sc.tile([prt, ng], FP, tag=f"d{tag}")
                    tt(dv, sv, rcv, ALU.mult)
                    return cv, dv

                c1, t_ = trig(h[0], f"1_{tidx}")   # t_ = tan(h1)
                c2, wm = trig(h[1], f"2_{tidx}")   # wm = tan(h2) = -w
                tm = sc.tile([prt, ng], FP, tag=f"tm{tidx}")
                nc.vector.tensor_scalar_mul(tm, t_, -1.0)
                w_ = sc.tile([prt, ng], FP, tag=f"w{tidx}")
                nc.vector.tensor_scalar_mul(w_, wm, -1.0)
                gg = tp.tile([prt, ng], FP, tag="gg")
                tt(gg, c1, c2, ALU.mult)
                # per-step product over the nq gates (pairwise tree; no mult-reduce)
                cur = gg.rearrange("p (a b) -> p a b", b=nq)
                n = nq
                lvl = 0
                while n > 1:
                    hn = n // 2
                    nxt = sc.tile([prt, seq, hn], FP, tag=f"gl{lvl}_{tidx}")
                    tt(nxt, cur[:, :, :hn], cur[:, :, hn:2 * hn], ALU.mult)
                    if n % 2:
                        tt(nxt[:, :, 0:1], nxt[:, :, 0:1], cur[:, :, n - 1:n], ALU.mult)
                    cur, n, lvl = nxt, hn, lvl + 1
                gcol = cur.rearrange("p a b -> p (a b)")

                # ---------- state init ----------
                Ar = st.tile([prt, dim], FP, tag=f"Ar{tidx}")
                Ai = st.tile([prt, dim], FP, tag=f"Ai{tidx}")
                Br = st.tile([prt, dim], FP, tag=f"Br{tidx}")
                Bi = st.tile([prt, dim], FP, tag=f"Bi{tidx}")
                nc.vector.memset(Ar, 0.0)
                nc.vector.memset(Ai, 0.0)
                nc.vector.memset(Ar[:, 0:1], 1.0)

                # ---------- evolution ----------
                for i in range(seq):
                    for j in range(nq):
                        k = i * nq + j
                        A0r, A1r = _halves(Ar, j, nq)
                        A0i, A1i = _halves(Ai, j, nq)
                        B0r, B1r = _halves(Br, j, nq)
                        B0i, B1i = _halves(Bi, j, nq)
                        tc_ = t_[:, k:k + 1]
                        tmc = tm[:, k:k + 1]
                        wc = w_[:, k:k + 1]
                        wmc = wm[:, k:k + 1]
                        # RY (Givens, pending scale cos(h1)): u = [[1,-t],[t,1]] psi
                        stt(B0r, A1r, tmc, A0r, ALU.mult, ALU.add)
                        stt(B0i, A1i, tmc, A0i, ALU.mult, ALU.add)
                        stt(B1r, A0r, tc_, A1r, ALU.mult, ALU.add)
                        stt(B1i, A0i, tc_, A1i, ALU.mult, ALU.add)
                        # RZ (pending scale cos(h2)): v0 = (1+iw)u0, v1 = (1-iw)u1
                        stt(A0r, B0i, wmc, B0r, ALU.mult, ALU.add)
                        stt(A0i, B0r, wc, B0i, ALU.mult, ALU.add)
                        stt(A1r, B1i, wc, B1r, ALU.mult, ALU.add)
                        stt(A1i, B1r, wmc, B1i, ALU.mult, ALU.add)
                    # CNOT ring: swap {c=1,t=0} <-> {c=1,t=1} via B scratch
                    for q in range(nq):
                        c, tq = (q, q + 1) if q < nq - 1 else (nq - 1, 0)
                        for (Apl, Tpl) in ((Ar, Br), (Ai, Bi)):
                            Xv, Yv = _cnot_xy(Apl, c, tq, nq)
                            Tv = Tpl[:, :dim // 4]
                            if len(Xv.shape) > 2:
                                free = Xv.shape[1:]
                                Tv = Tv.rearrange(
                                    "p (a m s) -> p a m s", a=free[0], m=free[1], s=free[2]
                                )
                            nc.gpsimd.tensor_copy(Tv, Xv)
                            nc.gpsimd.tensor_copy(Xv, Yv)
                            nc.gpsimd.tensor_copy(Yv, Tv)
                    # renorm the deferred per-gate scales (on ScalarE, hidden)
                    gc = gcol[:, i:i + 1]
                    nc.scalar.activation(Ar, Ar, AF.Copy, bias=0.0, scale=gc)
                    nc.scalar.activation(Ai, Ai, AF.Copy, bias=0.0, scale=gc)

                # ---------- observables ----------
                # p = |psi|^2 (into Br), then 3 signed halving trees (into Bi views)
                nc.scalar.activation(Br, Ar, AF.Square)
                nc.scalar.activation(Bi, Ai, AF.Square)
                tt(Br, Br, Bi, ALU.add)
                out_t = tp.tile([prt, 3], FP, tag="out")
                n3 = nq // 3
                for grp in range(3):
                    wires = range(grp * n3, (grp + 1) * n3)
                    cur = Br[:, :dim]
                    cur_n = dim
                    off = 0
                    for wi in wires:
                        s = 1 << (nq - 1 - wi)
                        a = cur_n // (2 * s)
                        v = cur.rearrange("p (a two s) -> p a two s", two=2, s=s, a=a)
                        nxt_n = cur_n // 2
                        nxt = Bi[:, off:off + nxt_n]
                        off += nxt_n
                        nv = nxt.rearrange("p (a s) -> p a s", a=a, s=s)
                        tt(nv, v[:, :, 0, :], v[:, :, 1, :], ALU.subtract)
                        cur = nxt
                        cur_n = nxt_n
                    ex = tp.tile([prt, 1], FP, tag="ex")
                    nc.vector.tensor_reduce(ex, cur, axis=AX, op=ALU.add)
                    # out = (exp + 1) / 2
                    nc.vector.tensor_scalar(
                        out_t[:, grp:grp + 1], ex, 0.5, 0.5, ALU.mult, ALU.add
                    )
                nc.sync.dma_start(
                    out=y_ext[tidx * prt:(tidx + 1) * prt], in_=out_t
                )

    nc.compile()
    _CACHE[key] = nc
    return nc


def kernel(x, weights, bias):
    from concourse.bass_utils import run_bass_kernel_spmd

    nc = build_program()
    in_maps = [
        {
            "x": np.ascontiguousarray(x[i * B_LOC:(i + 1) * B_LOC], dtype=np.float32),
            "w": np.ascontiguousarray(weights, dtype=np.float32),
            "b": np.ascontiguousarray(bias, dtype=np.float32),
        }
        for i in range(N_CORES)
    ]
    res = run_bass_kernel_spmd(nc, in_maps, list(range(N_CORES)))
    return np.concatenate([res.results[i]["y"] for i in range(N_CORES)], axis=0)
